# revision 11
# baseline (speedup 1.0000x reference)
"""Trainium2 Bass kernel for nn_ModelDecomposedExport_34213709480436.

AlphaZero-style dense transformer: B=32, 19x19 board (S=361), C=768, H=12,
HD=64, FFN=2048 (SwiGLU), L=8, 2D RoPE, conv stem, RMSNorm.

Strategy: pure data parallel over batch across 8 cores (4 batches/core, no
collectives). On-device activations are kept TRANSPOSED: [C on partitions
(6x128 tiles), (batch_pair=2, token=361) on free]. Heavy matmuls run in bf16
(full PE rate + fast-weight-load; ~5e-3 end-to-end rel err) with fp32 PSUM
accumulation; the residual stream stays fp32.

- Linear layers: out^T = W.T @ x^T with bf16 weights as lhsT.
- Conv stem folded into a matmul via host-side im2col (f32r for accuracy).
- Attention per (batch, head): scoresT[sk,sq] = k^T.T @ q^T with the two
  64-row head-halves interleaved (distinct PE row groups run concurrently);
  exp on ACT; o^T = [v|1].T @ expT gives o and softmax sums in one pass;
  1/sum via reciprocal_approx_fast + K=1 broadcast matmul.
- RoPE: partition-block shuffle via SBUF->SBUF DMA (cross-partition reads
  are illegal on DVE; gpsimd copies are slow) + sign-folded sin constant,
  all in bf16 (2 elem/cycle DVE modes).
- SwiGLU via AF.Silu activation + one vector mul from PSUM.
- FFN processed in thirds (SwiGLU pairing a/b chunks) to bound SBUF.
- Final rmsnorm in transposed layout (fp32), then PE-transpose to [S, C].
"""
import os
import sys
from contextlib import ExitStack

import ml_dtypes
import numpy as np

for _p in ("/opt/trn_rl_repo", "/root/.axon_site/_ro/trn_rl_repo"):
    try:
        import concourse  # noqa: F401
        break
    except ImportError:
        if os.path.isdir(_p) and _p not in sys.path:
            sys.path.insert(0, _p)

import concourse.bass as bass  # noqa: F401
import concourse.tile as tile
from concourse import bacc, mybir
from concourse.bass_utils import run_bass_kernel_spmd

F32 = mybir.dt.float32
F32R = mybir.dt.float32r
BF16 = mybir.dt.bfloat16

POS = 19
S = POS * POS          # 361
C = 768
H = 12
HD = 64
FFN = 2048
EPS = 1e-6
NCORES = 8

CT = C // 128           # 6 c-tiles
ST = [128, 128, 105]    # token tiles within one batch (sum=361)
ST_OFF = [0, 128, 256]
SP = S + 1            # f32r matmuls need an even moving dim; pad with a zero col
FT = [(0, 768), (768, 768), (1536, 512)]   # ffn-third (offset, width)
AF = mybir.ActivationFunctionType


# ----------------------------------------------------------------------------
# host-side input preparation
# ----------------------------------------------------------------------------

def _make_rope():
    d_half = HD // 2          # 32
    quarter = d_half // 2     # 16
    inv_freq = 1.0 / (10000.0 ** (np.arange(quarter, dtype=np.float64) / quarter))
    pos = np.arange(POS, dtype=np.float64)
    rows = np.repeat(pos, POS)
    cols = np.tile(pos, POS)
    ang_r = rows[:, None] * inv_freq[None, :]
    ang_c = cols[:, None] * inv_freq[None, :]
    emb = np.concatenate([ang_r, ang_c], axis=-1)      # [S, 32]
    emb = np.concatenate([emb, emb], axis=-1)          # [S, 64]
    return np.cos(emb).astype(np.float32), np.sin(emb).astype(np.float32)


def _rope_consts():
    cos, sin = _make_rope()            # [S, HD]
    cosT = cos.T                       # [HD, S]
    sinT = sin.T
    sgn = np.where(np.arange(HD) < HD // 2, -1.0, 1.0).astype(np.float32)
    sinTs = sinT * sgn[:, None]
    cos2 = np.tile(cosT, (2, 1))       # [128, S] (2 head copies)
    sin2 = np.tile(sinTs, (2, 1))
    pad = np.zeros((128, SP - S), np.float32)
    cos2 = np.concatenate([cos2, pad], axis=1)
    sin2 = np.concatenate([sin2, pad], axis=1)
    cos2 = np.repeat(cos2[:, None, :], 2, axis=1)      # [128,2,SP]
    sin2 = np.repeat(sin2[:, None, :], 2, axis=1)
    return (np.ascontiguousarray(cos2.astype(ml_dtypes.bfloat16)),
            np.ascontiguousarray(sin2.astype(ml_dtypes.bfloat16)))


def _stem_inputs(input_spatial, input_global):
    """im2col (SAME 3x3) + global rows -> A_ext [B, 217, S]."""
    B, CI = input_spatial.shape[:2]
    pad = np.zeros((B, CI, POS + 2, POS + 2), np.float32)
    pad[:, :, 1:-1, 1:-1] = input_spatial
    rows = []
    for ci in range(CI):
        for dy in range(3):
            for dx in range(3):
                rows.append(pad[:, ci, dy:dy + POS, dx:dx + POS].reshape(B, S))
    A = np.stack(rows, axis=1)                       # [B, 198, S]
    G = np.repeat(input_global[:, :, None], S, axis=2).astype(np.float32)
    return np.concatenate([A, G], axis=1)            # [B, 217, S]


def _stem_weights(conv_w, Wg):
    w = conv_w.transpose(1, 2, 3, 0).reshape(22 * 9, C)
    return np.ascontiguousarray(np.concatenate([w, Wg], axis=0).astype(np.float32))


def _bf(a):
    return np.ascontiguousarray(np.asarray(a, dtype=ml_dtypes.bfloat16))


def prepare_inputs(inputs, nb):
    """Build the per-core in_maps. nb = batches per core."""
    ins = {k: np.asarray(v, dtype=np.float32) for k, v in inputs.items()}
    B = ins["input_spatial"].shape[0]
    assert B == nb * NCORES, (B, nb)
    pairs = nb // 2

    a_full = _stem_inputs(ins["input_spatial"], ins["input_global"])  # [B,217,S]
    a_full = np.concatenate(
        [a_full, np.zeros((B, 217, SP - S), np.float32)], axis=-1)
    a_full = a_full.reshape(NCORES, pairs, 2, 217, SP).transpose(0, 1, 3, 2, 4)
    a_full = np.ascontiguousarray(a_full)

    w_stem = _stem_weights(ins["conv_w"], ins["Wg"])
    wqkv_f = ins["qkv_gamma"][:, :, None] * ins["Wqkv"]     # [L, C, 3C]
    wqkv = _bf(wqkv_f[:, :, :2 * C])        # q,k
    wv = _bf(wqkv_f[:, :, 2 * C:])          # v
    wfc1 = _bf(ins["mlp_gamma"][:, :, None] * ins["Wfc1"])
    cos2, sin2 = _rope_consts()
    gamma_f = np.ascontiguousarray(ins["final_gamma"].reshape(CT, 128).T)

    shared = {
        "w_stem": w_stem,
        "wqkv": wqkv,
        "wv": wv,
        "wproj": _bf(ins["Wproj"]),
        "wfc1": wfc1,
        "wfc2": _bf(ins["Wfc2"]),
        "cos2": cos2,
        "sin2s": sin2,
        "identity": np.eye(128, dtype=np.float32),
        "gamma_f": gamma_f.astype(np.float32),
        "ones_d": np.ones((128, 128), np.float32),
        "epsb_d": np.full((1, 1), EPS, np.float32),
        "ones_bf": np.ones((128, H), ml_dtypes.bfloat16),
    }
    in_maps = []
    for core in range(NCORES):
        m = dict(shared)
        m["a_ext"] = a_full[core]
        in_maps.append(m)
    return in_maps


# ----------------------------------------------------------------------------
# device program
# ----------------------------------------------------------------------------

def build_nc(L=8, NB=4):
    pairs = NB // 2
    nc = bacc.Bacc("TRN2", target_bir_lowering=False, debug=False)

    dr = {}
    dr["a_ext"] = nc.dram_tensor("a_ext", [pairs, 217, 2, SP], F32R,
                                 kind="ExternalInput").ap()
    dr["w_stem"] = nc.dram_tensor("w_stem", [217, C], F32R,
                                  kind="ExternalInput").ap()
    dr["wqkv"] = nc.dram_tensor("wqkv", [L, C, 2 * C], BF16,
                                kind="ExternalInput").ap()
    dr["wv"] = nc.dram_tensor("wv", [L, C, C], BF16, kind="ExternalInput").ap()
    dr["wproj"] = nc.dram_tensor("wproj", [L, C, C], BF16,
                                 kind="ExternalInput").ap()
    dr["wfc1"] = nc.dram_tensor("wfc1", [L, C, 2 * FFN], BF16,
                                kind="ExternalInput").ap()
    dr["wfc2"] = nc.dram_tensor("wfc2", [L, FFN, C], BF16,
                                kind="ExternalInput").ap()
    dr["cos2"] = nc.dram_tensor("cos2", [128, 2, SP], BF16,
                                kind="ExternalInput").ap()
    dr["sin2s"] = nc.dram_tensor("sin2s", [128, 2, SP], BF16,
                                 kind="ExternalInput").ap()
    dr["identity"] = nc.dram_tensor("identity", [128, 128], F32,
                                    kind="ExternalInput").ap()
    dr["gamma_f"] = nc.dram_tensor("gamma_f", [128, CT], F32,
                                   kind="ExternalInput").ap()
    dr["ones_d"] = nc.dram_tensor("ones_d", [128, 128], F32R,
                                  kind="ExternalInput").ap()
    dr["epsb_d"] = nc.dram_tensor("epsb_d", [1, 1], F32,
                                  kind="ExternalInput").ap()
    dr["ones_bf"] = nc.dram_tensor("ones_bf", [128, H], BF16,
                                   kind="ExternalInput").ap()
    dr["out"] = nc.dram_tensor("out", [NB, S, C], F32, kind="ExternalOutput").ap()

    with tile.TileContext(nc) as tc:
        with ExitStack() as ctx, nc.allow_low_precision(reason="bf16 pipeline"):
            _body(ctx, tc, L, pairs, dr)
    nc.compile()
    return nc


def _body(ctx, tc, L, pairs, dr):
    nc = tc.nc
    consts = ctx.enter_context(tc.tile_pool(name="consts", bufs=1))
    xp = ctx.enter_context(tc.tile_pool(name="xp", bufs=1))
    wp = ctx.enter_context(tc.tile_pool(name="wp", bufs=1))
    work = ctx.enter_context(tc.tile_pool(name="work", bufs=1))

    # ---- constants -------------------------------------------------------
    cos2 = consts.tile([128, 2, SP], BF16, name="cos2_sb")
    sin2 = consts.tile([128, 2, SP], BF16, name="sin2_sb")
    ident = consts.tile([128, 128], F32, name="ident_sb")
    gamma_f = consts.tile([128, CT], F32, name="gamma_sb")
    nc.sync.dma_start(cos2, dr["cos2"])
    nc.sync.dma_start(sin2, dr["sin2s"])
    nc.sync.dma_start(ident, dr["identity"])
    nc.sync.dma_start(gamma_f, dr["gamma_f"])
    epsb = consts.tile([1, 1], F32, name="epsb")
    ones_c = consts.tile([128, 1], F32R, name="ones_c")   # K-column of ones
    ones_r = consts.tile([1, 128], F32R, name="ones_r")   # bcast lhsT M=128
    ones_bf = consts.tile([128, H], BF16, name="ones_bf")
    nc.sync.dma_start(epsb, dr["epsb_d"])
    nc.sync.dma_start(ones_c, dr["ones_d"][:, 0:1])
    nc.sync.dma_start(ones_r, dr["ones_d"][0:1, :])
    nc.sync.dma_start(ones_bf, dr["ones_bf"])
    ones64 = ones_r[0:1, 0:64]                            # bcast lhsT M=64
    ones2 = consts.tile([128, 64], F32R, name="ones2")
    nc.sync.dma_start(ones2, dr["ones_d"][:, 0:64])

    # persistent x tiles (transposed activations, fp32)
    x = [[xp.tile([128, 2, SP], F32, name=f"x_{p}_{j}") for j in range(CT)]
         for p in range(pairs)]

    _wsn = [0]

    def load_wset(dram2d, row0, nrows, col0, width):
        tiles = []
        for kt in range(nrows // 128):
            _wsn[0] += 1
            t = wp.tile([128, width], BF16, tag="ws", bufs=14,
                        name=f"ws{_wsn[0]}")
            nc.sync.dma_start(
                t, dram2d[row0 + kt * 128: row0 + (kt + 1) * 128,
                          col0:col0 + width])
            tiles.append(t)
        return tiles

    def rms_norm(pp, xs, h_dtype, gamma_tile=None, out_tag="h"):
        """h = x * rsqrt(mean_c x^2 + eps) (+ optional per-partition gamma)."""
        sq = []
        for j in range(CT):
            t = work.tile([128, 2, SP], F32R, tag="sq", bufs=2, name=f"sq{j}")
            nc.scalar.square(t, xs[j])
            sq.append(t)
        ssq = pp.tile([128, 1024], F32, tag="mm", bufs=3, name="ssq_ps")
        for j in range(CT):
            for b in range(2):
                nc.tensor.matmul(ssq[0:1, b * 512:b * 512 + SP],
                                 ones_c, sq[j][:, b, :],
                                 start=(j == 0), stop=(j == CT - 1))
        lnv = work.tile([1, 2, SP], F32, tag="lnv", bufs=1, name="lnv")
        rinv = work.tile([1, 2, SP], F32R, tag="rinv", bufs=1, name="rinv")
        for b in range(2):
            nc.scalar.activation(lnv[0:1, b, :], ssq[0:1, b * 512:b * 512 + SP],
                                 AF.Ln, bias=epsb[0:1, 0:1], scale=1.0 / C)
        for b in range(2):
            nc.scalar.activation(rinv[0:1, b, :], lnv[0:1, b, :],
                                 AF.Exp, scale=-0.5)
        rps = pp.tile([128, 1024], F32, tag="mm", bufs=3, name="rps")
        for b in range(2):
            nc.tensor.matmul(rps[:, b * 512:b * 512 + SP], ones_r,
                             rinv[0:1, b, :], start=True, stop=True)
        hs = []
        rv = rps.rearrange("p (b n) -> p b n", b=2)[:, :, 0:SP]
        for j in range(CT):
            h = work.tile([128, 2, SP], h_dtype, tag=f"{out_tag}{j}",
                          bufs=2, name=f"{out_tag}{j}")
            if gamma_tile is None:
                nc.vector.tensor_mul(h, xs[j], rv)
            else:
                nc.vector.scalar_tensor_tensor(
                    h, xs[j], gamma_tile[:, j:j + 1], rv,
                    op0=mybir.AluOpType.mult, op1=mybir.AluOpType.mult)
            hs.append(h)
        return hs

    def bview(acc):
        return acc.rearrange("p (b n) -> p b n", b=2)[:, :, 0:SP]

    # ---- stem ------------------------------------------------------------
    ws_stem0 = wp.tile([128, C], F32R, tag="wstem", bufs=1, name="ws_stem0")
    nc.sync.dma_start(ws_stem0, dr["w_stem"][0:128, :])
    ws_stem1 = wp.tile([89, C], F32R, tag="wstem1", bufs=1, name="ws_stem1")
    nc.sync.dma_start(ws_stem1, dr["w_stem"][128:217, :])
    with tc.tile_pool(name="stem_sb", bufs=1) as sp, \
         tc.tile_pool(name="stem_ps", bufs=1, space="PSUM") as pp:
        for p in range(pairs):
            a0 = sp.tile([128, 2, SP], F32R, tag="a0", bufs=2, name="a0")
            a1 = sp.tile([89, 2, SP], F32R, tag="a1", bufs=2, name="a1")
            nc.sync.dma_start(a0, dr["a_ext"][p, 0:128])
            nc.sync.dma_start(a1, dr["a_ext"][p, 128:217])
            for j in range(CT):
                acc = pp.tile([128, 1024], F32, tag="mm", bufs=3, name="mmps")
                for b in range(2):
                    o = acc[:, b * 512:b * 512 + SP]
                    nc.tensor.matmul(o, ws_stem0[:, j * 128:(j + 1) * 128],
                                     a0[:, b, :], start=True, stop=False)
                    nc.tensor.matmul(o, ws_stem1[:, j * 128:(j + 1) * 128],
                                     a1[:, b, :], start=False, stop=True)
                nc.scalar.copy(x[p][j], bview(acc))

    # ---- layers ----------------------------------------------------------
    for li in range(L):
        for p in range(pairs):
            with ExitStack() as lctx:
                ap = lctx.enter_context(tc.tile_pool(name="atn_sb", bufs=1))
                pctx = ExitStack()
                pp = pctx.enter_context(
                    tc.tile_pool(name="lin_ps", bufs=1, space="PSUM"))
                hs = rms_norm(pp, x[p], BF16)

                # ---- QK^T (transposed) + rope ----------------------------
                wq = load_wset(dr["wqkv"][li], 0, C, 0, C)
                wk = load_wset(dr["wqkv"][li], 0, C, C, C)
                qk = []
                for m in range(2 * CT):
                    wset = wq if m < CT else wk
                    mj = m % CT
                    acc = pp.tile([128, 1024], F32, tag="mm", bufs=3,
                                  name="mmps")
                    for k in range(CT):
                        lhsT = wset[k][:, mj * 128:(mj + 1) * 128]
                        for b in range(2):
                            nc.tensor.matmul(acc[:, b * 512:b * 512 + SP],
                                             lhsT, hs[k][:, b, :],
                                             start=(k == 0), stop=(k == CT - 1))
                    raw = work.tile([128, 2, SP], BF16, tag="qraw", bufs=3,
                                    name="qraw")
                    nc.scalar.copy(raw, bview(acc))
                    shuf = work.tile([128, 2, SP], BF16, tag="qshuf", bufs=3,
                                     name="qshuf")
                    for g in range(4):
                        src = (g // 2) * 64 + (1 - g % 2) * 32
                        dst = (g // 2) * 64 + (g % 2) * 32
                        nc.sync.dma_start(shuf[dst:dst + 32],
                                          raw[src:src + 32])
                    m1 = work.tile([128, 2, SP], BF16, tag="ropeA", bufs=2,
                                   name="ropeA")
                    m2 = work.tile([128, 2, SP], BF16, tag="ropeB", bufs=2,
                                   name="ropeB")
                    nc.vector.tensor_mul(m1, raw, cos2)
                    nc.vector.tensor_mul(m2, shuf, sin2)
                    rot = ap.tile([128, 2, SP], BF16, tag=f"qk{m}",
                                  name=f"qk{m}")
                    nc.vector.tensor_add(rot, m1, m2)
                    qk.append(rot)

                # ---- V (natural layout, + ones column) -------------------
                wvs = load_wset(dr["wv"][li], 0, C, 0, C)
                vext = [ap.tile([128, H, 65], BF16, tag=f"vext{i}",
                                name=f"vext{i}") for i in range(6)]
                for b in range(2):
                    for mt in range(3):
                        sz = ST[mt]
                        acc = pp.tile([128, 1024], F32, tag="mm", bufs=3,
                                      name="mmps")
                        for k in range(CT):
                            lhsT = hs[k][:, b, ST_OFF[mt]:ST_OFF[mt] + sz]
                            nc.tensor.matmul(acc[0:sz, 0:512], lhsT,
                                             wvs[k][:, 0:512],
                                             start=(k == 0), stop=(k == CT - 1))
                            nc.tensor.matmul(acc[0:sz, 512:768], lhsT,
                                             wvs[k][:, 512:768],
                                             start=(k == 0), stop=(k == CT - 1))
                        vt = vext[b * 3 + mt]
                        with tc.high_priority():
                            nc.scalar.copy(vt[0:sz, :, 64:65],
                                           ones_bf[0:sz, :].unsqueeze(-1))
                            nc.scalar.copy(
                                vt[0:sz, :, 0:64],
                                acc[0:sz, 0:768].rearrange("p (h d) -> p h d",
                                                           h=H))

                # ---- attention -------------------------------------------
                wpr = load_wset(dr["wproj"][li], 0, C, 0, C)
                pctx.close()  # release lin_ps banks before attention psum
                with tc.tile_pool(name="atn_ps", bufs=1, space="PSUM") as app:
                    otiles = []
                    rrs = []
                    oraws = []
                    for jt in range(CT):
                        rr = work.tile([33, 2, SP], F32R, tag="rr",
                                       bufs=7, name="rr")
                        oraw = work.tile([128, 2, SP], BF16, tag="oraw",
                                         bufs=7, name="oraw")
                        rrs.append(rr)
                        oraws.append(oraw)
                        for b in range(2):
                            # both head-halves' scores back-to-back: the 64-row
                            # matmuls target distinct PE row groups and overlap
                            exs = [ap.tile([128, 3, SP], BF16, tag="expT",
                                           bufs=4, name="ex") for _ in (0, 1)]
                            for st in range(3):
                                sz = ST[st]
                                for half in (0, 1):
                                    hb = half * 64
                                    lhsT = qk[CT + jt][hb:hb + 64, b,
                                                       ST_OFF[st]:ST_OFF[st] + sz]
                                    rhs = qk[jt][hb:hb + 64, b, :]
                                    sc = app.tile([128, 512], F32, tag="sc",
                                                  bufs=4, name="sc")
                                    nc.tensor.matmul(
                                        sc[0:sz, 0:SP],
                                        lhsT, rhs, start=True, stop=True,
                                        tile_position=(hb, 0))
                                    nc.scalar.activation(
                                        exs[half][0:sz, st, :],
                                        sc[0:sz, 0:SP],
                                        AF.Exp, scale=1.0 / 8.0)
                            for half in (0, 1):
                                hb = half * 64
                                ex = exs[half]
                                ops_ = app.tile([128, 512], F32, tag="ops",
                                                bufs=2, name="ops")
                                hh = 2 * jt + half
                                for st in range(3):
                                    sz = ST[st]
                                    nc.tensor.matmul(
                                        ops_[0:65, 0:SP],
                                        vext[b * 3 + st][0:sz, hh, :],
                                        ex[0:sz, st, :],
                                        start=(st == 0), stop=(st == 2))
                                nc.vector.reciprocal(
                                    rr[half * 32:half * 32 + 1, b, :],
                                    ops_[64:65, 0:SP])
                                nc.scalar.copy(oraw[hb:hb + 64, b, :],
                                               ops_[0:64, 0:SP])
                    for jt in range(CT):
                        rr = rrs[jt]
                        oraw = oraws[jt]
                        ot = work.tile([128, 2, SP], BF16, tag=f"o{jt}",
                                       name=f"o{jt}")
                        for b in range(2):
                            for half in (0, 1):
                                hb = half * 64
                                rb = app.tile([64, 512], F32, tag="rbc",
                                              bufs=2, name="rbc")
                                nc.tensor.matmul(
                                    rb[0:64, 0:SP],
                                    ones2[half * 32:half * 32 + 1, 0:64],
                                    rr[half * 32:half * 32 + 1, b, :],
                                    start=True, stop=True)
                                nc.vector.tensor_mul(ot[hb:hb + 64, b, :],
                                                     oraw[hb:hb + 64, b, :],
                                                     rb[0:64, 0:SP])
                        otiles.append(ot)

                # ---- proj + residual -------------------------------------
                with tc.tile_pool(name="proj_ps", bufs=1, space="PSUM") as pj:
                    for m in range(CT):
                        acc = pj.tile([128, 1024], F32, tag="mm", bufs=3,
                                      name="mmps")
                        for k in range(CT):
                            lhsT = wpr[k][:, m * 128:(m + 1) * 128]
                            for b in range(2):
                                nc.tensor.matmul(
                                    acc[:, b * 512:b * 512 + SP],
                                    lhsT, otiles[k][:, b, :],
                                    start=(k == 0), stop=(k == CT - 1))
                        nc.vector.tensor_add(x[p][m], x[p][m], bview(acc))

            # ---- mlp (ffn thirds) ------------------------------------
            with tc.tile_pool(name="mlp_sb", bufs=1) as mp, \
                 tc.tile_pool(name="mlp_ps", bufs=1, space="PSUM") as pp:
                hs = rms_norm(pp, x[p], BF16)
                for (toff, tw) in FT:
                    wa = load_wset(dr["wfc1"][li], 0, C, toff, tw)
                    gt = []
                    for m in range(tw // 128):
                        acc = pp.tile([128, 1024], F32, tag="mm", bufs=3,
                                      name="mmps")
                        for k in range(CT):
                            lhsT = wa[k][:, m * 128:(m + 1) * 128]
                            for b in range(2):
                                nc.tensor.matmul(acc[:, b * 512:b * 512 + SP],
                                                 lhsT, hs[k][:, b, :],
                                                 start=(k == 0),
                                                 stop=(k == CT - 1))
                        sg = mp.tile([128, 2, SP], BF16, tag=f"sg{m}",
                                     name=f"sg{m}")
                        nc.scalar.activation(sg, bview(acc), AF.Silu)
                        gt.append(sg)
                    wb = load_wset(dr["wfc1"][li], 0, C, FFN + toff, tw)
                    gf = []
                    for m in range(tw // 128):
                        acc = pp.tile([128, 1024], F32, tag="mm", bufs=3,
                                      name="mmps")
                        for k in range(CT):
                            lhsT = wb[k][:, m * 128:(m + 1) * 128]
                            for b in range(2):
                                nc.tensor.matmul(acc[:, b * 512:b * 512 + SP],
                                                 lhsT, hs[k][:, b, :],
                                                 start=(k == 0),
                                                 stop=(k == CT - 1))
                        g = mp.tile([128, 2, SP], BF16, tag=f"g{m}",
                                    name=f"g{m}")
                        nc.vector.tensor_mul(g, gt[m], bview(acc))
                        gf.append(g)
                    w2 = load_wset(dr["wfc2"][li], toff, tw, 0, C)
                    for m in range(CT):
                        acc = pp.tile([128, 1024], F32, tag="mm", bufs=3,
                                      name="mmps")
                        nk = tw // 128
                        for k in range(nk):
                            lhsT = w2[k][:, m * 128:(m + 1) * 128]
                            for b in range(2):
                                nc.tensor.matmul(acc[:, b * 512:b * 512 + SP],
                                                 lhsT, gf[k][:, b, :],
                                                 start=(k == 0),
                                                 stop=(k == nk - 1))
                        nc.vector.tensor_add(x[p][m], x[p][m], bview(acc))

    # ---- final rmsnorm + transpose + output ------------------------------
    with tc.tile_pool(name="fin_sb", bufs=1) as fp, \
         tc.tile_pool(name="fin_ps", bufs=1, space="PSUM") as pp:
        for p in range(pairs):
            fin = rms_norm(pp, x[p], F32, gamma_tile=gamma_f, out_tag="h")
            for b in range(2):
                for st in range(3):
                    sz = ST[st]
                    onat = fp.tile([128, C], F32, tag="onat", bufs=3,
                                   name="onat")
                    for j in range(CT):
                        tp = pp.tile([128, 512], F32, tag="tp", bufs=2,
                                     name="tp")
                        nc.tensor.transpose(
                            tp[0:sz, 0:128],
                            fin[j][:, b, ST_OFF[st]:ST_OFF[st] + sz], ident)
                        nc.scalar.copy(onat[0:sz, j * 128:(j + 1) * 128],
                                       tp[0:sz, 0:128])
                    nc.sync.dma_start(
                        dr["out"][p * 2 + b, ST_OFF[st]:ST_OFF[st] + sz, :],
                        onat[0:sz, :])


# ----------------------------------------------------------------------------
# public entry point
# ----------------------------------------------------------------------------

_NC_CACHE = {}


def _get_nc(L, NB):
    key = (L, NB)
    if key not in _NC_CACHE:
        _NC_CACHE[key] = build_nc(L, NB)
    return _NC_CACHE[key]


def kernel(**inputs) -> np.ndarray:
    NB = np.asarray(inputs["input_spatial"]).shape[0] // NCORES
    L = np.asarray(inputs["Wqkv"]).shape[0]
    nc = _get_nc(L, NB)
    in_maps = prepare_inputs(inputs, NB)
    res = run_bass_kernel_spmd(nc, in_maps, list(range(NCORES)))
    return np.concatenate([r["out"] for r in res.results], axis=0)


# revision 15
# speedup vs baseline: 1.0030x; 1.0030x over previous
"""Trainium2 Bass kernel for nn_ModelDecomposedExport_34213709480436.

AlphaZero-style dense transformer: B=32, 19x19 board (S=361), C=768, H=12,
HD=64, FFN=2048 (SwiGLU), L=8, 2D RoPE, conv stem, RMSNorm.

Strategy: pure data parallel over batch across 8 cores (4 batches/core, no
collectives). On-device activations are kept TRANSPOSED: [C on partitions
(6x128 tiles), (batch_pair=2, token=361) on free]. Heavy matmuls run in bf16
(full PE rate + fast-weight-load; ~5e-3 end-to-end rel err) with fp32 PSUM
accumulation; the residual stream stays fp32.

- Linear layers: out^T = W.T @ x^T with bf16 weights as lhsT.
- Conv stem folded into a matmul via host-side im2col (f32r for accuracy).
- Attention per (batch, head): scoresT[sk,sq] = k^T.T @ q^T with the two
  64-row head-halves interleaved (distinct PE row groups run concurrently);
  exp on ACT; o^T = [v|1].T @ expT gives o and softmax sums in one pass;
  1/sum via reciprocal_approx_fast + K=1 broadcast matmul.
- RoPE: partition-block shuffle via SBUF->SBUF DMA (cross-partition reads
  are illegal on DVE; gpsimd copies are slow) + sign-folded sin constant,
  all in bf16 (2 elem/cycle DVE modes).
- SwiGLU via AF.Silu activation + one vector mul from PSUM.
- FFN processed in thirds (SwiGLU pairing a/b chunks) to bound SBUF.
- Final rmsnorm in transposed layout (fp32), then PE-transpose to [S, C].
"""
import os
import sys
from contextlib import ExitStack

import ml_dtypes
import numpy as np

for _p in ("/opt/trn_rl_repo", "/root/.axon_site/_ro/trn_rl_repo"):
    try:
        import concourse  # noqa: F401
        break
    except ImportError:
        if os.path.isdir(_p) and _p not in sys.path:
            sys.path.insert(0, _p)

import concourse.bass as bass  # noqa: F401
import concourse.tile as tile
from concourse import bacc, mybir
from concourse.bass_utils import run_bass_kernel_spmd

F32 = mybir.dt.float32
F32R = mybir.dt.float32r
BF16 = mybir.dt.bfloat16

POS = 19
S = POS * POS          # 361
C = 768
H = 12
HD = 64
FFN = 2048
EPS = 1e-6
NCORES = 8

CT = C // 128           # 6 c-tiles
ST = [128, 128, 105]    # token tiles within one batch (sum=361)
ST_OFF = [0, 128, 256]
SP = S + 1            # f32r matmuls need an even moving dim; pad with a zero col
FT = [(0, 768), (768, 768), (1536, 512)]   # ffn-third (offset, width)
AF = mybir.ActivationFunctionType


# ----------------------------------------------------------------------------
# host-side input preparation
# ----------------------------------------------------------------------------

def _make_rope():
    d_half = HD // 2          # 32
    quarter = d_half // 2     # 16
    inv_freq = 1.0 / (10000.0 ** (np.arange(quarter, dtype=np.float64) / quarter))
    pos = np.arange(POS, dtype=np.float64)
    rows = np.repeat(pos, POS)
    cols = np.tile(pos, POS)
    ang_r = rows[:, None] * inv_freq[None, :]
    ang_c = cols[:, None] * inv_freq[None, :]
    emb = np.concatenate([ang_r, ang_c], axis=-1)      # [S, 32]
    emb = np.concatenate([emb, emb], axis=-1)          # [S, 64]
    return np.cos(emb).astype(np.float32), np.sin(emb).astype(np.float32)


def _rope_consts():
    cos, sin = _make_rope()            # [S, HD]
    cosT = cos.T                       # [HD, S]
    sinT = sin.T
    sgn = np.where(np.arange(HD) < HD // 2, -1.0, 1.0).astype(np.float32)
    sinTs = sinT * sgn[:, None]
    cos2 = np.tile(cosT, (2, 1))       # [128, S] (2 head copies)
    sin2 = np.tile(sinTs, (2, 1))
    pad = np.zeros((128, SP - S), np.float32)
    cos2 = np.concatenate([cos2, pad], axis=1)
    sin2 = np.concatenate([sin2, pad], axis=1)
    cos2 = np.repeat(cos2[:, None, :], 2, axis=1)      # [128,2,SP]
    sin2 = np.repeat(sin2[:, None, :], 2, axis=1)
    return (np.ascontiguousarray(cos2.astype(ml_dtypes.bfloat16)),
            np.ascontiguousarray(sin2.astype(ml_dtypes.bfloat16)))


def _stem_inputs(input_spatial, input_global):
    """im2col (SAME 3x3) + global rows -> A_ext [B, 217, S]."""
    B, CI = input_spatial.shape[:2]
    pad = np.zeros((B, CI, POS + 2, POS + 2), np.float32)
    pad[:, :, 1:-1, 1:-1] = input_spatial
    rows = []
    for ci in range(CI):
        for dy in range(3):
            for dx in range(3):
                rows.append(pad[:, ci, dy:dy + POS, dx:dx + POS].reshape(B, S))
    A = np.stack(rows, axis=1)                       # [B, 198, S]
    G = np.repeat(input_global[:, :, None], S, axis=2).astype(np.float32)
    return np.concatenate([A, G], axis=1)            # [B, 217, S]


def _stem_weights(conv_w, Wg):
    w = conv_w.transpose(1, 2, 3, 0).reshape(22 * 9, C)
    return np.ascontiguousarray(np.concatenate([w, Wg], axis=0).astype(np.float32))


def _bf(a):
    return np.ascontiguousarray(np.asarray(a, dtype=ml_dtypes.bfloat16))


def prepare_inputs(inputs, nb):
    """Build the per-core in_maps. nb = batches per core."""
    ins = {k: np.asarray(v, dtype=np.float32) for k, v in inputs.items()}
    B = ins["input_spatial"].shape[0]
    assert B == nb * NCORES, (B, nb)
    pairs = nb // 2

    a_full = _stem_inputs(ins["input_spatial"], ins["input_global"])  # [B,217,S]
    a_full = np.concatenate(
        [a_full, np.zeros((B, 217, SP - S), np.float32)], axis=-1)
    a_full = a_full.reshape(NCORES, pairs, 2, 217, SP).transpose(0, 1, 3, 2, 4)
    a_full = np.ascontiguousarray(a_full)

    w_stem = _stem_weights(ins["conv_w"], ins["Wg"])
    wqkv_f = ins["qkv_gamma"][:, :, None] * ins["Wqkv"]     # [L, C, 3C]
    wqkv = _bf(wqkv_f[:, :, :2 * C])        # q,k
    wv = _bf(wqkv_f[:, :, 2 * C:])          # v
    wfc1 = _bf(ins["mlp_gamma"][:, :, None] * ins["Wfc1"])
    cos2, sin2 = _rope_consts()
    gamma_f = np.ascontiguousarray(ins["final_gamma"].reshape(CT, 128).T)

    shared = {
        "w_stem": w_stem,
        "wqkv": wqkv,
        "wv": wv,
        "wproj": _bf(ins["Wproj"]),
        "wfc1": wfc1,
        "wfc2": _bf(ins["Wfc2"]),
        "cos2": cos2,
        "sin2s": sin2,
        "identity": np.eye(128, dtype=np.float32),
        "gamma_f": gamma_f.astype(np.float32),
        "ones_d": np.ones((128, 128), np.float32),
        "epsb_d": np.full((1, 1), EPS, np.float32),
        "ones_bf": np.ones((128, H), ml_dtypes.bfloat16),
    }
    in_maps = []
    for core in range(NCORES):
        m = dict(shared)
        m["a_ext"] = a_full[core]
        in_maps.append(m)
    return in_maps


# ----------------------------------------------------------------------------
# device program
# ----------------------------------------------------------------------------

def build_nc(L=8, NB=4):
    pairs = NB // 2
    nc = bacc.Bacc("TRN2", target_bir_lowering=False, debug=False)

    dr = {}
    dr["a_ext"] = nc.dram_tensor("a_ext", [pairs, 217, 2, SP], F32R,
                                 kind="ExternalInput").ap()
    dr["w_stem"] = nc.dram_tensor("w_stem", [217, C], F32R,
                                  kind="ExternalInput").ap()
    dr["wqkv"] = nc.dram_tensor("wqkv", [L, C, 2 * C], BF16,
                                kind="ExternalInput").ap()
    dr["wv"] = nc.dram_tensor("wv", [L, C, C], BF16, kind="ExternalInput").ap()
    dr["wproj"] = nc.dram_tensor("wproj", [L, C, C], BF16,
                                 kind="ExternalInput").ap()
    dr["wfc1"] = nc.dram_tensor("wfc1", [L, C, 2 * FFN], BF16,
                                kind="ExternalInput").ap()
    dr["wfc2"] = nc.dram_tensor("wfc2", [L, FFN, C], BF16,
                                kind="ExternalInput").ap()
    dr["cos2"] = nc.dram_tensor("cos2", [128, 2, SP], BF16,
                                kind="ExternalInput").ap()
    dr["sin2s"] = nc.dram_tensor("sin2s", [128, 2, SP], BF16,
                                 kind="ExternalInput").ap()
    dr["identity"] = nc.dram_tensor("identity", [128, 128], F32,
                                    kind="ExternalInput").ap()
    dr["gamma_f"] = nc.dram_tensor("gamma_f", [128, CT], F32,
                                   kind="ExternalInput").ap()
    dr["ones_d"] = nc.dram_tensor("ones_d", [128, 128], F32R,
                                  kind="ExternalInput").ap()
    dr["epsb_d"] = nc.dram_tensor("epsb_d", [1, 1], F32,
                                  kind="ExternalInput").ap()
    dr["ones_bf"] = nc.dram_tensor("ones_bf", [128, H], BF16,
                                   kind="ExternalInput").ap()
    dr["out"] = nc.dram_tensor("out", [NB, S, C], F32, kind="ExternalOutput").ap()

    with tile.TileContext(nc) as tc:
        with ExitStack() as ctx, nc.allow_low_precision(reason="bf16 pipeline"):
            _body(ctx, tc, L, pairs, dr)
    nc.compile()
    return nc


def _body(ctx, tc, L, pairs, dr):
    nc = tc.nc
    consts = ctx.enter_context(tc.tile_pool(name="consts", bufs=1))
    xp = ctx.enter_context(tc.tile_pool(name="xp", bufs=1))
    wp = ctx.enter_context(tc.tile_pool(name="wp", bufs=1))
    work = ctx.enter_context(tc.tile_pool(name="work", bufs=1))

    # ---- constants -------------------------------------------------------
    cos2 = consts.tile([128, 2, SP], BF16, name="cos2_sb")
    sin2 = consts.tile([128, 2, SP], BF16, name="sin2_sb")
    ident = consts.tile([128, 128], F32, name="ident_sb")
    gamma_f = consts.tile([128, CT], F32, name="gamma_sb")
    nc.sync.dma_start(cos2, dr["cos2"])
    nc.sync.dma_start(sin2, dr["sin2s"])
    nc.sync.dma_start(ident, dr["identity"])
    nc.sync.dma_start(gamma_f, dr["gamma_f"])
    epsb = consts.tile([1, 1], F32, name="epsb")
    ones_c = consts.tile([128, 1], F32R, name="ones_c")   # K-column of ones
    ones_r = consts.tile([1, 128], F32R, name="ones_r")   # bcast lhsT M=128
    ones_bf = consts.tile([128, H], BF16, name="ones_bf")
    nc.sync.dma_start(epsb, dr["epsb_d"])
    nc.sync.dma_start(ones_c, dr["ones_d"][:, 0:1])
    nc.sync.dma_start(ones_r, dr["ones_d"][0:1, :])
    nc.sync.dma_start(ones_bf, dr["ones_bf"])
    ones64 = ones_r[0:1, 0:64]                            # bcast lhsT M=64
    ones2 = consts.tile([128, 64], F32R, name="ones2")
    nc.sync.dma_start(ones2, dr["ones_d"][:, 0:64])

    # persistent x tiles (transposed activations, fp32)
    x = [[xp.tile([128, 2, SP], F32, name=f"x_{p}_{j}") for j in range(CT)]
         for p in range(pairs)]

    _wsn = [0]

    def load_wset(dram2d, row0, nrows, col0, width, tag="ws", bufs=20):
        tiles = []
        for kt in range(nrows // 128):
            _wsn[0] += 1
            t = wp.tile([128, width], BF16, tag=tag, bufs=bufs,
                        name=f"ws{_wsn[0]}")
            nc.sync.dma_start(
                t, dram2d[row0 + kt * 128: row0 + (kt + 1) * 128,
                          col0:col0 + width])
            tiles.append(t)
        return tiles

    def rms_norm(pp, xs, h_dtype, gamma_tile=None, out_tag="h"):
        """h = x * rsqrt(mean_c x^2 + eps) (+ optional per-partition gamma)."""
        sq = []
        for j in range(CT):
            t = work.tile([128, 2, SP], F32R, tag="sq", bufs=1, name=f"sq{j}")
            nc.scalar.square(t, xs[j])
            sq.append(t)
        ssq = pp.tile([128, 1024], F32, tag="mm", bufs=3, name="ssq_ps")
        for j in range(CT):
            for b in range(2):
                nc.tensor.matmul(ssq[0:1, b * 512:b * 512 + SP],
                                 ones_c, sq[j][:, b, :],
                                 start=(j == 0), stop=(j == CT - 1))
        lnv = work.tile([1, 2, SP], F32, tag="lnv", bufs=1, name="lnv")
        rinv = work.tile([1, 2, SP], F32R, tag="rinv", bufs=1, name="rinv")
        for b in range(2):
            nc.scalar.activation(lnv[0:1, b, :], ssq[0:1, b * 512:b * 512 + SP],
                                 AF.Ln, bias=epsb[0:1, 0:1], scale=1.0 / C)
        for b in range(2):
            nc.scalar.activation(rinv[0:1, b, :], lnv[0:1, b, :],
                                 AF.Exp, scale=-0.5)
        rps = pp.tile([128, 1024], F32, tag="mm", bufs=3, name="rps")
        for b in range(2):
            nc.tensor.matmul(rps[:, b * 512:b * 512 + SP], ones_r,
                             rinv[0:1, b, :], start=True, stop=True)
        hs = []
        rv = rps.rearrange("p (b n) -> p b n", b=2)[:, :, 0:SP]
        for j in range(CT):
            h = work.tile([128, 2, SP], h_dtype, tag=f"{out_tag}{j}",
                          bufs=2, name=f"{out_tag}{j}")
            if gamma_tile is None:
                nc.vector.tensor_mul(h, xs[j], rv)
            else:
                nc.vector.scalar_tensor_tensor(
                    h, xs[j], gamma_tile[:, j:j + 1], rv,
                    op0=mybir.AluOpType.mult, op1=mybir.AluOpType.mult)
            hs.append(h)
        return hs

    def bview(acc):
        return acc.rearrange("p (b n) -> p b n", b=2)[:, :, 0:SP]

    # ---- stem ------------------------------------------------------------
    with tc.tile_pool(name="stem_sb", bufs=1) as sp, \
         tc.tile_pool(name="stem_ps", bufs=1, space="PSUM") as pp:
        ws_stem0 = sp.tile([128, C], F32R, tag="wstem", bufs=1, name="ws_stem0")
        nc.sync.dma_start(ws_stem0, dr["w_stem"][0:128, :])
        ws_stem1 = sp.tile([89, C], F32R, tag="wstem1", bufs=1, name="ws_stem1")
        nc.sync.dma_start(ws_stem1, dr["w_stem"][128:217, :])
        for p in range(pairs):
            a0 = sp.tile([128, 2, SP], F32R, tag="a0", bufs=2, name="a0")
            a1 = sp.tile([89, 2, SP], F32R, tag="a1", bufs=2, name="a1")
            nc.sync.dma_start(a0, dr["a_ext"][p, 0:128])
            nc.sync.dma_start(a1, dr["a_ext"][p, 128:217])
            for j in range(CT):
                acc = pp.tile([128, 1024], F32, tag="mm", bufs=3, name="mmps")
                for b in range(2):
                    o = acc[:, b * 512:b * 512 + SP]
                    nc.tensor.matmul(o, ws_stem0[:, j * 128:(j + 1) * 128],
                                     a0[:, b, :], start=True, stop=False)
                    nc.tensor.matmul(o, ws_stem1[:, j * 128:(j + 1) * 128],
                                     a1[:, b, :], start=False, stop=True)
                nc.scalar.copy(x[p][j], bview(acc))

    # ---- layers ----------------------------------------------------------
    # Per layer: issue [rms+qkv+rope+V] for BOTH pairs first (weights loaded
    # once), then [attention+proj] per pair, then [mlp] per pair. The dense
    # GEMMs of the other pair fill the PE while attention waits on
    # exp/reciprocal latency on the scalar/vector engines.
    for li in range(L):
        with ExitStack() as lctx:
            ap = lctx.enter_context(tc.tile_pool(name="atn_sb", bufs=1))
            qk_all = []
            vext_all = []
            with tc.tile_pool(name="lin_ps", bufs=1, space="PSUM") as pp:
                wq = load_wset(dr["wqkv"][li], 0, C, 0, C)
                wk = load_wset(dr["wqkv"][li], 0, C, C, C)
                wvs = load_wset(dr["wv"][li], 0, C, 0, C)
                for p in range(pairs):
                    hs = rms_norm(pp, x[p], BF16)

                    # ---- QK^T (transposed) + rope ------------------------
                    qk = []
                    for m in range(2 * CT):
                        wset = wq if m < CT else wk
                        mj = m % CT
                        acc = pp.tile([128, 1024], F32, tag="mm", bufs=3,
                                      name="mmps")
                        for k in range(CT):
                            lhsT = wset[k][:, mj * 128:(mj + 1) * 128]
                            for b in range(2):
                                nc.tensor.matmul(
                                    acc[:, b * 512:b * 512 + SP],
                                    lhsT, hs[k][:, b, :],
                                    start=(k == 0), stop=(k == CT - 1))
                        raw = work.tile([128, 2, SP], BF16, tag="qraw",
                                        bufs=2, name="qraw")
                        nc.scalar.copy(raw, bview(acc))
                        shuf = work.tile([128, 2, SP], BF16, tag="qshuf",
                                         bufs=2, name="qshuf")
                        for g in range(4):
                            src = (g // 2) * 64 + (1 - g % 2) * 32
                            dst = (g // 2) * 64 + (g % 2) * 32
                            nc.sync.dma_start(shuf[dst:dst + 32],
                                              raw[src:src + 32])
                        m1 = work.tile([128, 2, SP], BF16, tag="ropeA",
                                       bufs=2, name="ropeA")
                        m2 = work.tile([128, 2, SP], BF16, tag="ropeB",
                                       bufs=2, name="ropeB")
                        nc.vector.tensor_mul(m1, raw, cos2)
                        nc.vector.tensor_mul(m2, shuf, sin2)
                        rot = ap.tile([128, 2, SP], BF16, tag=f"qk{p}_{m}",
                                      name=f"qk{p}_{m}")
                        nc.vector.tensor_add(rot, m1, m2)
                        qk.append(rot)
                    qk_all.append(qk)

                    # ---- V (natural layout, + ones column) ---------------
                    vext = [ap.tile([128, H, 65], BF16, tag=f"vext{p}_{i}",
                                    name=f"vext{p}_{i}") for i in range(6)]
                    for b in range(2):
                        for mt in range(3):
                            sz = ST[mt]
                            acc = pp.tile([128, 1024], F32, tag="mm", bufs=3,
                                          name="mmps")
                            for k in range(CT):
                                lhsT = hs[k][:, b, ST_OFF[mt]:ST_OFF[mt] + sz]
                                nc.tensor.matmul(acc[0:sz, 0:512], lhsT,
                                                 wvs[k][:, 0:512],
                                                 start=(k == 0),
                                                 stop=(k == CT - 1))
                                nc.tensor.matmul(acc[0:sz, 512:768], lhsT,
                                                 wvs[k][:, 512:768],
                                                 start=(k == 0),
                                                 stop=(k == CT - 1))
                            vt = vext[b * 3 + mt]
                            with tc.high_priority():
                                nc.scalar.copy(vt[0:sz, :, 64:65],
                                               ones_bf[0:sz, :].unsqueeze(-1))
                                nc.scalar.copy(
                                    vt[0:sz, :, 0:64],
                                    acc[0:sz, 0:768].rearrange(
                                        "p (h d) -> p h d", h=H))
                    vext_all.append(vext)

            wpr = load_wset(dr["wproj"][li], 0, C, 0, C)

            # ---- attention + proj, per pair ------------------------------
            for p in range(pairs):
                qk = qk_all[p]
                vext = vext_all[p]
                with tc.tile_pool(name="atn_ps", bufs=1, space="PSUM") as app:
                    otiles = []
                    rrs = []
                    oraws = []
                    for jt in range(CT):
                        rr = work.tile([33, 2, SP], F32R, tag="rr",
                                       bufs=4, name="rr")
                        oraw = work.tile([128, 2, SP], BF16, tag="oraw",
                                         bufs=4, name="oraw")
                        rrs.append(rr)
                        oraws.append(oraw)
                        for b in range(2):
                            # both head-halves' scores back-to-back: distinct
                            # PE row groups overlap in the array
                            exs = [ap.tile([128, 3, SP], BF16, tag="expT",
                                           bufs=2, name="ex") for _ in (0, 1)]
                            for st in range(3):
                                sz = ST[st]
                                for half in (0, 1):
                                    hb = half * 64
                                    lhsT = qk[CT + jt][hb:hb + 64, b,
                                                       ST_OFF[st]:ST_OFF[st] + sz]
                                    rhs = qk[jt][hb:hb + 64, b, :]
                                    sc = app.tile([128, 512], F32, tag="sc",
                                                  bufs=4, name="sc")
                                    nc.tensor.matmul(
                                        sc[0:sz, 0:SP],
                                        lhsT, rhs, start=True, stop=True,
                                        tile_position=(hb, 0))
                                    nc.scalar.activation(
                                        exs[half][0:sz, st, :],
                                        sc[0:sz, 0:SP],
                                        AF.Exp, scale=1.0 / 8.0)
                            for half in (0, 1):
                                hb = half * 64
                                ex = exs[half]
                                ops_ = app.tile([128, 512], F32, tag="ops",
                                                bufs=2, name="ops")
                                hh = 2 * jt + half
                                for st in range(3):
                                    sz = ST[st]
                                    nc.tensor.matmul(
                                        ops_[0:65, 0:SP],
                                        vext[b * 3 + st][0:sz, hh, :],
                                        ex[0:sz, st, :],
                                        start=(st == 0), stop=(st == 2))
                                nc.vector.reciprocal(
                                    rr[half * 32:half * 32 + 1, b, :],
                                    ops_[64:65, 0:SP])
                                nc.scalar.copy(oraw[hb:hb + 64, b, :],
                                               ops_[0:64, 0:SP])
                        if jt % 3 == 2:
                            for jw in range(jt - 2, jt + 1):
                                rrw = rrs[jw]
                                orw = oraws[jw]
                                ot = work.tile([128, 2, SP], BF16,
                                               tag=f"o{jw}", name=f"o{jw}")
                                for b in range(2):
                                    for half in (0, 1):
                                        hb = half * 64
                                        rb = app.tile([64, 512], F32,
                                                      tag="rbc", bufs=2,
                                                      name="rbc")
                                        nc.tensor.matmul(
                                            rb[0:64, 0:SP],
                                            ones2[half * 32:half * 32 + 1,
                                                  0:64],
                                            rrw[half * 32:half * 32 + 1, b, :],
                                            start=True, stop=True)
                                        nc.vector.tensor_mul(
                                            ot[hb:hb + 64, b, :],
                                            orw[hb:hb + 64, b, :],
                                            rb[0:64, 0:SP])
                                otiles.append(ot)

                # ---- proj + residual ---------------------------------
                with tc.tile_pool(name="proj_ps", bufs=1, space="PSUM") as pj:
                    for m in range(CT):
                        acc = pj.tile([128, 1024], F32, tag="mm", bufs=3,
                                      name="mmps")
                        for k in range(CT):
                            lhsT = wpr[k][:, m * 128:(m + 1) * 128]
                            for b in range(2):
                                nc.tensor.matmul(
                                    acc[:, b * 512:b * 512 + SP],
                                    lhsT, otiles[k][:, b, :],
                                    start=(k == 0), stop=(k == CT - 1))
                        nc.vector.tensor_add(x[p][m], x[p][m], bview(acc))

        # ---- mlp (ffn thirds), per pair ----------------------------------
        for p in range(pairs):
            with tc.tile_pool(name="mlp_sb", bufs=1) as mp, \
                 tc.tile_pool(name="mlp_ps", bufs=1, space="PSUM") as pp:
                hs = rms_norm(pp, x[p], BF16)
                for (toff, tw) in FT:
                    wa = load_wset(dr["wfc1"][li], 0, C, toff, tw)
                    gt = []
                    for m in range(tw // 128):
                        acc = pp.tile([128, 1024], F32, tag="mm", bufs=3,
                                      name="mmps")
                        for k in range(CT):
                            lhsT = wa[k][:, m * 128:(m + 1) * 128]
                            for b in range(2):
                                nc.tensor.matmul(acc[:, b * 512:b * 512 + SP],
                                                 lhsT, hs[k][:, b, :],
                                                 start=(k == 0),
                                                 stop=(k == CT - 1))
                        sg = mp.tile([128, 2, SP], BF16, tag=f"sg{m}",
                                     name=f"sg{m}")
                        nc.scalar.activation(sg, bview(acc), AF.Silu)
                        gt.append(sg)
                    wb = load_wset(dr["wfc1"][li], 0, C, FFN + toff, tw)
                    gf = []
                    for m in range(tw // 128):
                        acc = pp.tile([128, 1024], F32, tag="mm", bufs=3,
                                      name="mmps")
                        for k in range(CT):
                            lhsT = wb[k][:, m * 128:(m + 1) * 128]
                            for b in range(2):
                                nc.tensor.matmul(acc[:, b * 512:b * 512 + SP],
                                                 lhsT, hs[k][:, b, :],
                                                 start=(k == 0),
                                                 stop=(k == CT - 1))
                        g = mp.tile([128, 2, SP], BF16, tag=f"g{m}",
                                    name=f"g{m}")
                        nc.vector.tensor_mul(g, gt[m], bview(acc))
                        gf.append(g)
                    w2 = load_wset(dr["wfc2"][li], toff, tw, 0, C)
                    for m in range(CT):
                        acc = pp.tile([128, 1024], F32, tag="mm", bufs=3,
                                      name="mmps")
                        nk = tw // 128
                        for k in range(nk):
                            lhsT = w2[k][:, m * 128:(m + 1) * 128]
                            for b in range(2):
                                nc.tensor.matmul(acc[:, b * 512:b * 512 + SP],
                                                 lhsT, gf[k][:, b, :],
                                                 start=(k == 0),
                                                 stop=(k == nk - 1))
                        nc.vector.tensor_add(x[p][m], x[p][m], bview(acc))

    # ---- final rmsnorm + transpose + output ------------------------------
    with tc.tile_pool(name="fin_sb", bufs=1) as fp, \
         tc.tile_pool(name="fin_ps", bufs=1, space="PSUM") as pp:
        for p in range(pairs):
            fin = rms_norm(pp, x[p], F32, gamma_tile=gamma_f, out_tag="h")
            for b in range(2):
                for st in range(3):
                    sz = ST[st]
                    onat = fp.tile([128, C], F32, tag="onat", bufs=3,
                                   name="onat")
                    for j in range(CT):
                        tp = pp.tile([128, 512], F32, tag="tp", bufs=2,
                                     name="tp")
                        nc.tensor.transpose(
                            tp[0:sz, 0:128],
                            fin[j][:, b, ST_OFF[st]:ST_OFF[st] + sz], ident)
                        nc.scalar.copy(onat[0:sz, j * 128:(j + 1) * 128],
                                       tp[0:sz, 0:128])
                    nc.sync.dma_start(
                        dr["out"][p * 2 + b, ST_OFF[st]:ST_OFF[st] + sz, :],
                        onat[0:sz, :])


# ----------------------------------------------------------------------------
# public entry point
# ----------------------------------------------------------------------------

_NC_CACHE = {}


def _get_nc(L, NB):
    key = (L, NB)
    if key not in _NC_CACHE:
        _NC_CACHE[key] = build_nc(L, NB)
    return _NC_CACHE[key]


def kernel(**inputs) -> np.ndarray:
    NB = np.asarray(inputs["input_spatial"]).shape[0] // NCORES
    L = np.asarray(inputs["Wqkv"]).shape[0]
    nc = _get_nc(L, NB)
    in_maps = prepare_inputs(inputs, NB)
    res = run_bass_kernel_spmd(nc, in_maps, list(range(NCORES)))
    return np.concatenate([r["out"] for r in res.results], axis=0)


# revision 21
# speedup vs baseline: 1.0686x; 1.0654x over previous
"""Trainium2 Bass kernel for nn_ModelDecomposedExport_34213709480436.

AlphaZero-style dense transformer: B=32, 19x19 board (S=361), C=768, H=12,
HD=64, FFN=2048 (SwiGLU), L=8, 2D RoPE, conv stem, RMSNorm.

Strategy: pure data parallel over batch across 8 cores (4 batches/core, no
collectives). On-device activations are kept TRANSPOSED: [C on partitions
(6x128 tiles), (batch_pair=2, token=361) on free]. Heavy matmuls run in bf16
(full PE rate + fast-weight-load; ~5e-3 end-to-end rel err) with fp32 PSUM
accumulation; the residual stream stays fp32.

- Linear layers: out^T = W.T @ x^T with bf16 weights as lhsT.
- Conv stem folded into a matmul via host-side im2col (f32r for accuracy).
- Attention per (batch, head): scoresT[sk,sq] = k^T.T @ q^T with the two
  64-row head-halves interleaved (distinct PE row groups run concurrently);
  exp on ACT; o^T = [v|1].T @ expT gives o and softmax sums in one pass;
  1/sum via reciprocal_approx_fast + K=1 broadcast matmul.
- RoPE: partition-block shuffle via SBUF->SBUF DMA (cross-partition reads
  are illegal on DVE; gpsimd copies are slow) + sign-folded sin constant,
  all in bf16 (2 elem/cycle DVE modes).
- SwiGLU via AF.Silu activation + one vector mul from PSUM.
- FFN processed in thirds (SwiGLU pairing a/b chunks) to bound SBUF.
- Final rmsnorm in transposed layout (fp32), then PE-transpose to [S, C].
"""
import os
import sys
from contextlib import ExitStack

import ml_dtypes
import numpy as np

for _p in ("/opt/trn_rl_repo", "/root/.axon_site/_ro/trn_rl_repo"):
    try:
        import concourse  # noqa: F401
        break
    except ImportError:
        if os.path.isdir(_p) and _p not in sys.path:
            sys.path.insert(0, _p)

import concourse.bass as bass  # noqa: F401
import concourse.tile as tile
from concourse import bacc, mybir
from concourse.bass_utils import run_bass_kernel_spmd

F32 = mybir.dt.float32
F32R = mybir.dt.float32r
BF16 = mybir.dt.bfloat16

POS = 19
S = POS * POS          # 361
C = 768
H = 12
HD = 64
FFN = 2048
EPS = 1e-6
NCORES = 8

CT = C // 128           # 6 c-tiles
ST = [128, 128, 105]    # token tiles within one batch (sum=361)
ST_OFF = [0, 128, 256]
SP = S + 1            # f32r matmuls need an even moving dim; pad with a zero col
FT = [(0, 768), (768, 768), (1536, 512)]   # ffn-third (offset, width)
AF = mybir.ActivationFunctionType


# ----------------------------------------------------------------------------
# host-side input preparation
# ----------------------------------------------------------------------------

def _make_rope():
    d_half = HD // 2          # 32
    quarter = d_half // 2     # 16
    inv_freq = 1.0 / (10000.0 ** (np.arange(quarter, dtype=np.float64) / quarter))
    pos = np.arange(POS, dtype=np.float64)
    rows = np.repeat(pos, POS)
    cols = np.tile(pos, POS)
    ang_r = rows[:, None] * inv_freq[None, :]
    ang_c = cols[:, None] * inv_freq[None, :]
    emb = np.concatenate([ang_r, ang_c], axis=-1)      # [S, 32]
    emb = np.concatenate([emb, emb], axis=-1)          # [S, 64]
    return np.cos(emb).astype(np.float32), np.sin(emb).astype(np.float32)


def _rope_consts():
    cos, sin = _make_rope()            # [S, HD]
    cosT = cos.T                       # [HD, S]
    sinT = sin.T
    sgn = np.where(np.arange(HD) < HD // 2, -1.0, 1.0).astype(np.float32)
    sinTs = sinT * sgn[:, None]
    cos2 = np.tile(cosT, (2, 1))       # [128, S] (2 head copies)
    sin2 = np.tile(sinTs, (2, 1))
    pad = np.zeros((128, SP - S), np.float32)
    cos2 = np.concatenate([cos2, pad], axis=1)
    sin2 = np.concatenate([sin2, pad], axis=1)
    cos2 = np.repeat(cos2[:, None, :], 2, axis=1)      # [128,2,SP]
    sin2 = np.repeat(sin2[:, None, :], 2, axis=1)
    return (np.ascontiguousarray(cos2.astype(ml_dtypes.bfloat16)),
            np.ascontiguousarray(sin2.astype(ml_dtypes.bfloat16)))


def _stem_inputs(input_spatial, input_global):
    """im2col (SAME 3x3) + global rows -> A_ext [B, 217, S]."""
    B, CI = input_spatial.shape[:2]
    pad = np.zeros((B, CI, POS + 2, POS + 2), np.float32)
    pad[:, :, 1:-1, 1:-1] = input_spatial
    rows = []
    for ci in range(CI):
        for dy in range(3):
            for dx in range(3):
                rows.append(pad[:, ci, dy:dy + POS, dx:dx + POS].reshape(B, S))
    A = np.stack(rows, axis=1)                       # [B, 198, S]
    G = np.repeat(input_global[:, :, None], S, axis=2).astype(np.float32)
    return np.concatenate([A, G], axis=1)            # [B, 217, S]


def _stem_weights(conv_w, Wg):
    w = conv_w.transpose(1, 2, 3, 0).reshape(22 * 9, C)
    return np.ascontiguousarray(np.concatenate([w, Wg], axis=0).astype(np.float32))


def _bf(a):
    return np.ascontiguousarray(np.asarray(a, dtype=ml_dtypes.bfloat16))


def prepare_inputs(inputs, nb):
    """Build the per-core in_maps. nb = batches per core."""
    ins = {k: np.asarray(v, dtype=np.float32) for k, v in inputs.items()}
    B = ins["input_spatial"].shape[0]
    assert B == nb * NCORES, (B, nb)
    pairs = nb // 2

    a_full = _stem_inputs(ins["input_spatial"], ins["input_global"])  # [B,217,S]
    a_full = np.concatenate(
        [a_full, np.zeros((B, 217, SP - S), np.float32)], axis=-1)
    a_full = a_full.reshape(NCORES, pairs, 2, 217, SP).transpose(0, 1, 3, 2, 4)
    a_full = np.ascontiguousarray(a_full)

    w_stem = _stem_weights(ins["conv_w"], ins["Wg"])
    wqkv_f = ins["qkv_gamma"][:, :, None] * ins["Wqkv"]     # [L, C, 3C]
    wqkv = _bf(wqkv_f[:, :, :2 * C])        # q,k
    wv = _bf(wqkv_f[:, :, 2 * C:])          # v
    wfc1 = _bf(ins["mlp_gamma"][:, :, None] * ins["Wfc1"])
    cos2, sin2 = _rope_consts()
    gamma_f = np.ascontiguousarray(ins["final_gamma"].reshape(CT, 128).T)

    shared = {
        "w_stem": w_stem,
        "wqkv": wqkv,
        "wv": wv,
        "wproj": _bf(ins["Wproj"]),
        "wfc1": wfc1,
        "wfc2": _bf(ins["Wfc2"]),
        "cos2": cos2,
        "sin2s": sin2,
        "identity": np.eye(128, dtype=np.float32),
        "gamma_f": gamma_f.astype(np.float32),
        "ones_d": np.ones((128, 128), np.float32),
        "epsb_d": np.full((1, 1), EPS, np.float32),
        "ones_bf": np.ones((128, 64 * H), ml_dtypes.bfloat16),
    }
    in_maps = []
    for core in range(NCORES):
        m = dict(shared)
        m["a_ext"] = a_full[core]
        in_maps.append(m)
    return in_maps


# ----------------------------------------------------------------------------
# device program
# ----------------------------------------------------------------------------

def build_nc(L=8, NB=4):
    pairs = NB // 2
    nc = bacc.Bacc("TRN2", target_bir_lowering=False, debug=False)

    dr = {}
    dr["a_ext"] = nc.dram_tensor("a_ext", [pairs, 217, 2, SP], F32R,
                                 kind="ExternalInput").ap()
    dr["w_stem"] = nc.dram_tensor("w_stem", [217, C], F32R,
                                  kind="ExternalInput").ap()
    dr["wqkv"] = nc.dram_tensor("wqkv", [L, C, 2 * C], BF16,
                                kind="ExternalInput").ap()
    dr["wv"] = nc.dram_tensor("wv", [L, C, C], BF16, kind="ExternalInput").ap()
    dr["wproj"] = nc.dram_tensor("wproj", [L, C, C], BF16,
                                 kind="ExternalInput").ap()
    dr["wfc1"] = nc.dram_tensor("wfc1", [L, C, 2 * FFN], BF16,
                                kind="ExternalInput").ap()
    dr["wfc2"] = nc.dram_tensor("wfc2", [L, FFN, C], BF16,
                                kind="ExternalInput").ap()
    dr["cos2"] = nc.dram_tensor("cos2", [128, 2, SP], BF16,
                                kind="ExternalInput").ap()
    dr["sin2s"] = nc.dram_tensor("sin2s", [128, 2, SP], BF16,
                                 kind="ExternalInput").ap()
    dr["identity"] = nc.dram_tensor("identity", [128, 128], F32,
                                    kind="ExternalInput").ap()
    dr["gamma_f"] = nc.dram_tensor("gamma_f", [128, CT], F32,
                                   kind="ExternalInput").ap()
    dr["ones_d"] = nc.dram_tensor("ones_d", [128, 128], F32R,
                                  kind="ExternalInput").ap()
    dr["epsb_d"] = nc.dram_tensor("epsb_d", [1, 1], F32,
                                  kind="ExternalInput").ap()
    dr["ones_bf"] = nc.dram_tensor("ones_bf", [128, 64 * H], BF16,
                                   kind="ExternalInput").ap()
    dr["out"] = nc.dram_tensor("out", [NB, S, C], F32, kind="ExternalOutput").ap()

    with tile.TileContext(nc) as tc:
        with ExitStack() as ctx, nc.allow_low_precision(reason="bf16 pipeline"):
            _body(ctx, tc, L, pairs, dr)
    nc.compile()
    return nc


def _body(ctx, tc, L, pairs, dr):
    nc = tc.nc
    consts = ctx.enter_context(tc.tile_pool(name="consts", bufs=1))
    xp = ctx.enter_context(tc.tile_pool(name="xp", bufs=1))
    wp = ctx.enter_context(tc.tile_pool(name="wp", bufs=1))
    work = ctx.enter_context(tc.tile_pool(name="work", bufs=1))

    # ---- constants -------------------------------------------------------
    cos2 = consts.tile([128, 2, SP], BF16, name="cos2_sb")
    sin2 = consts.tile([128, 2, SP], BF16, name="sin2_sb")
    ident = consts.tile([128, 128], F32, name="ident_sb")
    gamma_f = consts.tile([128, CT], F32, name="gamma_sb")
    nc.sync.dma_start(cos2, dr["cos2"])
    nc.sync.dma_start(sin2, dr["sin2s"])
    nc.sync.dma_start(ident, dr["identity"])
    nc.sync.dma_start(gamma_f, dr["gamma_f"])
    epsb = consts.tile([1, 1], F32, name="epsb")
    ones_c = consts.tile([128, 1], F32R, name="ones_c")   # K-column of ones
    ones_r = consts.tile([1, 128], F32R, name="ones_r")   # bcast lhsT M=128
    ones_bf = consts.tile([128, 64 * H], BF16, name="ones_bf")
    nc.sync.dma_start(epsb, dr["epsb_d"])
    nc.sync.dma_start(ones_c, dr["ones_d"][:, 0:1])
    nc.sync.dma_start(ones_r, dr["ones_d"][0:1, :])
    nc.sync.dma_start(ones_bf, dr["ones_bf"])
    ones64 = ones_r[0:1, 0:64]                            # bcast lhsT M=64
    ones2 = consts.tile([128, 64], F32R, name="ones2")
    nc.sync.dma_start(ones2, dr["ones_d"][:, 0:64])

    # persistent x tiles (transposed activations, fp32)
    x = [[xp.tile([128, 2, SP], F32, name=f"x_{p}_{j}") for j in range(CT)]
         for p in range(pairs)]

    _wsn = [0]

    def load_wset(dram2d, row0, nrows, col0, width):
        tiles = []
        for kt in range(nrows // 128):
            _wsn[0] += 1
            t = wp.tile([128, width], BF16, tag="ws", bufs=14,
                        name=f"ws{_wsn[0]}")
            nc.sync.dma_start(
                t, dram2d[row0 + kt * 128: row0 + (kt + 1) * 128,
                          col0:col0 + width])
            tiles.append(t)
        return tiles

    def rms_norm(pp, xs, h_dtype, gamma_tile=None, out_tag="h"):
        """h = x * rsqrt(mean_c x^2 + eps) (+ optional per-partition gamma)."""
        sq = []
        for j in range(CT):
            t = work.tile([128, 2, SP], F32R, tag="sq", bufs=1, name=f"sq{j}")
            nc.scalar.square(t, xs[j])
            sq.append(t)
        ssq = pp.tile([128, 1024], F32, tag="mm", bufs=3, name="ssq_ps")
        for j in range(CT):
            for b in range(2):
                nc.tensor.matmul(ssq[0:1, b * 512:b * 512 + SP],
                                 ones_c, sq[j][:, b, :],
                                 start=(j == 0), stop=(j == CT - 1))
        lnv = work.tile([1, 2, SP], F32, tag="lnv", bufs=1, name="lnv")
        rinv = work.tile([1, 2, SP], F32R, tag="rinv", bufs=1, name="rinv")
        for b in range(2):
            nc.scalar.activation(lnv[0:1, b, :], ssq[0:1, b * 512:b * 512 + SP],
                                 AF.Ln, bias=epsb[0:1, 0:1], scale=1.0 / C)
        for b in range(2):
            nc.scalar.activation(rinv[0:1, b, :], lnv[0:1, b, :],
                                 AF.Exp, scale=-0.5)
        rps = pp.tile([128, 1024], F32, tag="mm", bufs=3, name="rps")
        for b in range(2):
            nc.tensor.matmul(rps[:, b * 512:b * 512 + SP], ones_r,
                             rinv[0:1, b, :], start=True, stop=True)
        hs = []
        rv = rps.rearrange("p (b n) -> p b n", b=2)[:, :, 0:SP]
        for j in range(CT):
            h = work.tile([128, 2, SP], h_dtype, tag=f"{out_tag}{j}",
                          bufs=2, name=f"{out_tag}{j}")
            if gamma_tile is None:
                nc.vector.tensor_mul(h, xs[j], rv)
            else:
                nc.vector.scalar_tensor_tensor(
                    h, xs[j], gamma_tile[:, j:j + 1], rv,
                    op0=mybir.AluOpType.mult, op1=mybir.AluOpType.mult)
            hs.append(h)
        return hs

    def bview(acc):
        return acc.rearrange("p (b n) -> p b n", b=2)[:, :, 0:SP]

    # ---- stem ------------------------------------------------------------
    ws_stem0 = wp.tile([128, C], F32R, tag="wstem", bufs=1, name="ws_stem0")
    nc.sync.dma_start(ws_stem0, dr["w_stem"][0:128, :])
    ws_stem1 = wp.tile([89, C], F32R, tag="wstem1", bufs=1, name="ws_stem1")
    nc.sync.dma_start(ws_stem1, dr["w_stem"][128:217, :])
    with tc.tile_pool(name="stem_sb", bufs=1) as sp, \
         tc.tile_pool(name="stem_ps", bufs=1, space="PSUM") as pp:
        for p in range(pairs):
            a0 = sp.tile([128, 2, SP], F32R, tag="a0", bufs=2, name="a0")
            a1 = sp.tile([89, 2, SP], F32R, tag="a1", bufs=2, name="a1")
            nc.sync.dma_start(a0, dr["a_ext"][p, 0:128])
            nc.sync.dma_start(a1, dr["a_ext"][p, 128:217])
            for j in range(CT):
                acc = pp.tile([128, 1024], F32, tag="mm", bufs=3, name="mmps")
                for b in range(2):
                    o = acc[:, b * 512:b * 512 + SP]
                    nc.tensor.matmul(o, ws_stem0[:, j * 128:(j + 1) * 128],
                                     a0[:, b, :], start=True, stop=False)
                    nc.tensor.matmul(o, ws_stem1[:, j * 128:(j + 1) * 128],
                                     a1[:, b, :], start=False, stop=True)
                nc.scalar.copy(x[p][j], bview(acc))

    # ---- layers ----------------------------------------------------------
    for li in range(L):
        for p in range(pairs):
            with ExitStack() as lctx:
                ap = lctx.enter_context(tc.tile_pool(name="atn_sb", bufs=1))
                pctx = ExitStack()
                pp = pctx.enter_context(
                    tc.tile_pool(name="lin_ps", bufs=1, space="PSUM"))
                hs = rms_norm(pp, x[p], BF16)

                # ---- QK^T (transposed) + rope ----------------------------
                wq = load_wset(dr["wqkv"][li], 0, C, 0, C)
                wk = load_wset(dr["wqkv"][li], 0, C, C, C)
                qk = []
                for m in range(2 * CT):
                    wset = wq if m < CT else wk
                    mj = m % CT
                    acc = pp.tile([128, 1024], F32, tag="mm", bufs=3,
                                  name="mmps")
                    for k in range(CT):
                        lhsT = wset[k][:, mj * 128:(mj + 1) * 128]
                        for b in range(2):
                            nc.tensor.matmul(acc[:, b * 512:b * 512 + SP],
                                             lhsT, hs[k][:, b, :],
                                             start=(k == 0), stop=(k == CT - 1))
                    raw = work.tile([128, 2, SP], BF16, tag="qraw", bufs=2,
                                    name="qraw")
                    nc.scalar.copy(raw, bview(acc))
                    shuf = work.tile([128, 2, SP], BF16, tag="qshuf", bufs=2,
                                     name="qshuf")
                    for g in range(4):
                        src = (g // 2) * 64 + (1 - g % 2) * 32
                        dst = (g // 2) * 64 + (g % 2) * 32
                        nc.sync.dma_start(shuf[dst:dst + 32],
                                          raw[src:src + 32])
                    m1 = work.tile([128, 2, SP], BF16, tag="ropeA", bufs=2,
                                   name="ropeA")
                    m2 = work.tile([128, 2, SP], BF16, tag="ropeB", bufs=1,
                                   name="ropeB")
                    nc.vector.tensor_mul(m1, raw, cos2)
                    nc.vector.tensor_mul(m2, shuf, sin2)
                    rot = ap.tile([128, 2, SP], BF16, tag=f"qk{m}",
                                  name=f"qk{m}")
                    nc.vector.tensor_add(rot, m1, m2)
                    qk.append(rot)

                # ---- V (natural layout, + ones column) -------------------
                wvs = load_wset(dr["wv"][li], 0, C, 0, C)
                vext = [ap.tile([128, H, 128], BF16, tag=f"vext{i}",
                                name=f"vext{i}") for i in range(6)]
                for b in range(2):
                    for mt in range(3):
                        sz = ST[mt]
                        acc = pp.tile([128, 1024], F32, tag="mm", bufs=3,
                                      name="mmps")
                        for k in range(CT):
                            lhsT = hs[k][:, b, ST_OFF[mt]:ST_OFF[mt] + sz]
                            nc.tensor.matmul(acc[0:sz, 0:512], lhsT,
                                             wvs[k][:, 0:512],
                                             start=(k == 0), stop=(k == CT - 1))
                            nc.tensor.matmul(acc[0:sz, 512:768], lhsT,
                                             wvs[k][:, 512:768],
                                             start=(k == 0), stop=(k == CT - 1))
                        vt = vext[b * 3 + mt]
                        with tc.high_priority():
                            nc.scalar.copy(
                                vt[0:sz, :, 0:64],
                                ones_bf[0:sz, :].rearrange(
                                    "p (h c) -> p h c", h=H))
                            nc.scalar.copy(
                                vt[0:sz, :, 64:128],
                                acc[0:sz, 0:768].rearrange("p (h d) -> p h d",
                                                           h=H))

                # ---- attention -------------------------------------------
                wpr = load_wset(dr["wproj"][li], 0, C, 0, C)
                pctx.close()  # release lin_ps banks before attention psum
                with tc.tile_pool(name="atn_ps", bufs=1, space="PSUM") as app:
                    otiles = []
                    rrs = []
                    oraws = []
                    for jt in range(CT):
                        rr = work.tile([33, 2, SP], F32R, tag="rr",
                                       bufs=7, name="rr")
                        oraw = work.tile([128, 2, SP], BF16, tag="oraw",
                                         bufs=7, name="oraw")
                        rrs.append(rr)
                        oraws.append(oraw)
                        for b in range(2):
                            # both head-halves' scores back-to-back: the 64-row
                            # matmuls target distinct PE row groups and overlap
                            exs = [ap.tile([128, 3, SP], BF16, tag="expT",
                                           bufs=4, name="ex") for _ in (0, 1)]
                            for st in range(3):
                                sz = ST[st]
                                for half in (0, 1):
                                    hb = half * 64
                                    lhsT = qk[CT + jt][hb:hb + 64, b,
                                                       ST_OFF[st]:ST_OFF[st] + sz]
                                    rhs = qk[jt][hb:hb + 64, b, :]
                                    sc = app.tile([128, 512], F32, tag="sc",
                                                  bufs=4, name="sc")
                                    nc.tensor.matmul(
                                        sc[0:sz, 0:SP],
                                        lhsT, rhs, start=True, stop=True,
                                        tile_position=(hb, 0))
                                    nc.scalar.activation(
                                        exs[half][0:sz, st, :],
                                        sc[0:sz, 0:SP],
                                        AF.Exp, scale=1.0 / 8.0)
                            for half in (0, 1):
                                hb = half * 64
                                ex = exs[half]
                                ops_ = app.tile([128, 512], F32, tag="ops",
                                                bufs=2, name="ops")
                                hh = 2 * jt + half
                                for st in range(3):
                                    sz = ST[st]
                                    nc.tensor.matmul(
                                        ops_[0:128, 0:SP],
                                        vext[b * 3 + st][0:sz, hh, :],
                                        ex[0:sz, st, :],
                                        start=(st == 0), stop=(st == 2))
                                rrf = work.tile([1, 2, SP], F32, tag="rrf",
                                                bufs=2, name="rrf")
                                nc.vector.reciprocal_approx_fast(
                                    rrf[0:1, b, :], ops_[0:1, 0:SP])
                                nc.gpsimd.tensor_copy(
                                    rr[half * 32:half * 32 + 1, b, :],
                                    rrf[0:1, b, :])
                                nc.scalar.copy(oraw[hb:hb + 64, b, :],
                                               ops_[64:128, 0:SP])
                    for jt in range(CT):
                        rr = rrs[jt]
                        oraw = oraws[jt]
                        ot = work.tile([128, 2, SP], BF16, tag=f"o{jt}",
                                       name=f"o{jt}")
                        for b in range(2):
                            for half in (0, 1):
                                hb = half * 64
                                rb = app.tile([64, 512], F32, tag="rbc",
                                              bufs=2, name="rbc")
                                nc.tensor.matmul(
                                    rb[0:64, 0:SP],
                                    ones2[half * 32:half * 32 + 1, 0:64],
                                    rr[half * 32:half * 32 + 1, b, :],
                                    start=True, stop=True)
                                nc.vector.tensor_mul(ot[hb:hb + 64, b, :],
                                                     oraw[hb:hb + 64, b, :],
                                                     rb[0:64, 0:SP])
                        otiles.append(ot)

                # ---- proj + residual -------------------------------------
                with tc.tile_pool(name="proj_ps", bufs=1, space="PSUM") as pj:
                    for m in range(CT):
                        acc = pj.tile([128, 1024], F32, tag="mm", bufs=3,
                                      name="mmps")
                        for k in range(CT):
                            lhsT = wpr[k][:, m * 128:(m + 1) * 128]
                            for b in range(2):
                                nc.tensor.matmul(
                                    acc[:, b * 512:b * 512 + SP],
                                    lhsT, otiles[k][:, b, :],
                                    start=(k == 0), stop=(k == CT - 1))
                        nc.vector.tensor_add(x[p][m], x[p][m], bview(acc))

            # ---- mlp (ffn thirds) ------------------------------------
            with tc.tile_pool(name="mlp_sb", bufs=1) as mp, \
                 tc.tile_pool(name="mlp_ps", bufs=1, space="PSUM") as pp:
                hs = rms_norm(pp, x[p], BF16)
                for (toff, tw) in FT:
                    wa = load_wset(dr["wfc1"][li], 0, C, toff, tw)
                    gt = []
                    for m in range(tw // 128):
                        acc = pp.tile([128, 1024], F32, tag="mm", bufs=3,
                                      name="mmps")
                        for k in range(CT):
                            lhsT = wa[k][:, m * 128:(m + 1) * 128]
                            for b in range(2):
                                nc.tensor.matmul(acc[:, b * 512:b * 512 + SP],
                                                 lhsT, hs[k][:, b, :],
                                                 start=(k == 0),
                                                 stop=(k == CT - 1))
                        sg = mp.tile([128, 2, SP], BF16, tag=f"sg{m}",
                                     name=f"sg{m}")
                        nc.scalar.activation(sg, bview(acc), AF.Silu)
                        gt.append(sg)
                    wb = load_wset(dr["wfc1"][li], 0, C, FFN + toff, tw)
                    gf = []
                    for m in range(tw // 128):
                        acc = pp.tile([128, 1024], F32, tag="mm", bufs=3,
                                      name="mmps")
                        for k in range(CT):
                            lhsT = wb[k][:, m * 128:(m + 1) * 128]
                            for b in range(2):
                                nc.tensor.matmul(acc[:, b * 512:b * 512 + SP],
                                                 lhsT, hs[k][:, b, :],
                                                 start=(k == 0),
                                                 stop=(k == CT - 1))
                        g = mp.tile([128, 2, SP], BF16, tag=f"g{m}",
                                    name=f"g{m}")
                        nc.vector.tensor_mul(g, gt[m], bview(acc))
                        gf.append(g)
                    w2 = load_wset(dr["wfc2"][li], toff, tw, 0, C)
                    for m in range(CT):
                        acc = pp.tile([128, 1024], F32, tag="mm", bufs=3,
                                      name="mmps")
                        nk = tw // 128
                        for k in range(nk):
                            lhsT = w2[k][:, m * 128:(m + 1) * 128]
                            for b in range(2):
                                nc.tensor.matmul(acc[:, b * 512:b * 512 + SP],
                                                 lhsT, gf[k][:, b, :],
                                                 start=(k == 0),
                                                 stop=(k == nk - 1))
                        nc.vector.tensor_add(x[p][m], x[p][m], bview(acc))

    # ---- final rmsnorm + transpose + output ------------------------------
    with tc.tile_pool(name="fin_sb", bufs=1) as fp, \
         tc.tile_pool(name="fin_ps", bufs=1, space="PSUM") as pp:
        for p in range(pairs):
            fin = rms_norm(pp, x[p], F32, gamma_tile=gamma_f, out_tag="h")
            for b in range(2):
                for st in range(3):
                    sz = ST[st]
                    onat = fp.tile([128, C], F32, tag="onat", bufs=3,
                                   name="onat")
                    for j in range(CT):
                        tp = pp.tile([128, 512], F32, tag="tp", bufs=2,
                                     name="tp")
                        nc.tensor.transpose(
                            tp[0:sz, 0:128],
                            fin[j][:, b, ST_OFF[st]:ST_OFF[st] + sz], ident)
                        nc.scalar.copy(onat[0:sz, j * 128:(j + 1) * 128],
                                       tp[0:sz, 0:128])
                    nc.sync.dma_start(
                        dr["out"][p * 2 + b, ST_OFF[st]:ST_OFF[st] + sz, :],
                        onat[0:sz, :])


# ----------------------------------------------------------------------------
# public entry point
# ----------------------------------------------------------------------------

_NC_CACHE = {}


def _get_nc(L, NB):
    key = (L, NB)
    if key not in _NC_CACHE:
        _NC_CACHE[key] = build_nc(L, NB)
    return _NC_CACHE[key]


def kernel(**inputs) -> np.ndarray:
    NB = np.asarray(inputs["input_spatial"]).shape[0] // NCORES
    L = np.asarray(inputs["Wqkv"]).shape[0]
    nc = _get_nc(L, NB)
    in_maps = prepare_inputs(inputs, NB)
    res = run_bass_kernel_spmd(nc, in_maps, list(range(NCORES)))
    return np.concatenate([r["out"] for r in res.results], axis=0)
